# revision 2
# baseline (speedup 1.0000x reference)
"""Trainium2 Bass kernel for nn_Encoder_62740882260638 (ragged set encoder), v3.

Pure data parallel over 8 NeuronCores: each core handles B/8 = 1024 sets.

v3 on top of v2: HWDGE dispatch is a single shared device (~625ns per DMA),
so DMA COUNT is a first-order cost. v3 cuts ~39 DMAs/chunk to ~15:
  * one x load per chunk ([128, 16, 256] in a single DMA)
  * persistent R2g (rhs one-hots) / KRZg (combined lhsT) double buffers with
    the static rows (i32 one-hots, KRt/Kds tables) written once at startup
  * rank one-hots: ONE broadcast DMA writes rank rows + mask row of R2g;
    the is_equal / is_le builds run on DVE after phase 3 (no queue
    head-blocking on Pool/DVE)
  * zpT scatter: one contiguous DMA per sub-chunk (lhsT cols reordered to
    [vm | km | kds])
  * em ones-row via a K=1 matmul slab instead of a per-chunk DMA
  * weight slabs loaded with one DMA per tensor
"""

from contextlib import ExitStack

import numpy as np

import concourse.bass as bass
import concourse.mybir as mybir
import concourse.tile as tile
from concourse import bacc
from concourse import bass_utils

FP = mybir.dt.float32
FPR = mybir.dt.float32r
BF = mybir.dt.bfloat16
I32 = mybir.dt.int32
AF = mybir.ActivationFunctionType
OP = mybir.AluOpType
AX = mybir.AxisListType

B, N, DIM, HID, MAXN1 = 8192, 16, 256, 512, 17
NCORES = 8
SC = B // NCORES  # sets per core (1024)
CS = 128          # sets per chunk
CT = CS * N       # tokens per chunk (2048)
NSUB = 512        # tokens per sub-chunk (matmul N)
BIG = 1.0e30
USE_GPS = True  # run elementwise offload ops on Pool (gpsimd) vs DVE
KC = 392 + HID + DIM  # KRZg col layout: [vm 0:512 | km 512:904 | kds 904:1160]


def _ksplit(total):
    return [(a, min(a + 128, total)) for a in range(0, total, 128)]


def build_program(sets_per_core=SC, num_devices=1):
    nc = bacc.Bacc(
        "TRN2", target_bir_lowering=False, debug=False,
        num_devices=num_devices,
    )
    S = sets_per_core
    assert S % CS == 0
    nchunks = S // CS

    def din(name, shape, dtype=FP):
        return nc.dram_tensor(name, shape, dtype, kind="ExternalInput").ap()

    x_d = din("x", [S * N, DIM])
    n_d = din("n_i", [S], I32)

    vdW1_SHAPE_MARK = [DIM, DIM], FPR)
    b1d = din("b1d", [DIM, 1])
    vdW2_SHAPE_MARK = [DIM, DIM], FPR)
    b2d = din("b2d", [DIM, 1])
    vmW1x_SHAPE_MARK = [DIM, HID], FPR)
    b1v = din("b1v", [HID, 1])
    vmW2_SHAPE_MARK = [HID, HID], FPR)
    b2v = din("b2v", [HID, 1])
    kmW2e_SHAPE_MARK = [393, HID], FPR)
    edW1_SHAPE_MARK = [DIM, DIM], FPR)
    edb1 = din("edb1", [DIM, 1])
    edW2_SHAPE_MARK = [DIM, DIM], FPR)
    Wze_SHAPE_MARK = [DIM + 1, 904], FPR)
    emW1y_SHAPE_MARK = [HID, 520], FPR)
    EMt_SHAPE_MARK = [MAXN1, 520], FPR)
    emW2_SHAPE_MARK = [520, HID], FPR)
    emOnes_SHAPE_MARK = [1, HID], FPR)
    rankWb = din("rankWb", [128, DIM])
    identB = din("identB", [128, 128], BF)
    KRTm = din("KRTm", [MAXN1, 392], BF)
    KDSm = din("KDSm", [MAXN1, DIM], BF)
    I32s4 = din("I32s4", [32, CT], BF)
    REP16 = din("REP16", [16, 128])
    SEL0 = din("SEL0", [16, 128])
    SEL1 = din("SEL1", [16, 128])
    S8 = din("S8", [128, 8], BF)
    LT0 = din("LT0", [128, 1])
    LT1 = din("LT1", [128, 1])
    iotaR2 = din("iotaR2", [50, 1])
    iota17 = din("iota17", [32, 1])
    iota8_0 = din("iota8_0", [8, 1])
    iota8_1 = din("iota8_1", [8, 1])
    onesb = din("onesb", [1, CS], BF)

    z_d = nc.dram_tensor("z_out", [S, HID], FP, kind="ExternalOutput").ap()

    with tile.TileContext(nc) as tc, ExitStack() as ctx, \
            nc.allow_low_precision(reason="bf16 activations within tolerance"):
        wpool = ctx.enter_context(tc.tile_pool(name="wpool", bufs=1))
        glob = ctx.enter_context(tc.tile_pool(name="glob", bufs=2))
        work = ctx.enter_context(tc.tile_pool(name="work", bufs=2))
        work3 = ctx.enter_context(tc.tile_pool(name="work3", bufs=2))
        ps = ctx.enter_context(tc.tile_pool(name="ps", bufs=1, space="PSUM"))
        dstg = ctx.enter_context(
            tc.tile_pool(name="dstg", bufs=2, space="DRAM"))

        def wload(ap, name):
            """Load a [K, M] weight as <=128-partition K-slabs, ONE DMA."""
            k = ap.shape[0]
            if len(ap.shape) == 1 or k <= 128:
                t = wpool.tile(ap.shape, ap.dtype, name=f"w_{name}")
                nc.sync.dma_start(out=t, in_=ap)
                return t
            nfull = k // 128
            rem = k - 128 * nfull
            m = ap.shape[1]
            t = wpool.tile([128, nfull, m], ap.dtype, name=f"w_{name}")
            nc.sync.dma_start(
                out=t,
                in_=ap[0:128 * nfull].rearrange("(q p) m -> p q m", p=128))
            slabs = [t[:, q, :] for q in range(nfull)]
            if rem:
                tr = wpool.tile([rem, m], ap.dtype, name=f"w_{name}_r")
                nc.sync.dma_start(out=tr, in_=ap[128 * nfull:])
                slabs.append(tr)
            return slabs

        # persistent double-buffered rhs/lhsT tiles for the one-hot matmuls;
        # static rows written once at startup, dynamic rows cycled per chunk.
        #   R2g rows: 0-31 i32 set one-hots (static), 32-48 rank one-hots;
        #             maskt = valid-token mask row. cols = CT tokens.
        #   KRZg rows: 0-31 zpT slab (per chunk), 32-48 KRt|Kds (static).
        #             free = [4 sub-chunks, KC cols].
        R2g = [wpool.tile([49, CT], BF, name=f"R2g{i}") for i in range(2)]
        maskt = [wpool.tile([1, CT], BF, name=f"maskt{i}") for i in range(2)]
        KRZg = [wpool.tile([49, 4, KC], BF, name=f"KRZg{i}") for i in range(2)]

        def load_small_early():
            return (wload(rankWb, "rankWb"), wload(iota17, "iota17"))

        def load_small_rest():
            r = (wload(REP16, "REP16"),
                 [wload(SEL0, "SEL0"), wload(SEL1, "SEL1")],
                 wload(S8, "S8"),
                 [wload(LT0, "LT0"), wload(LT1, "LT1")],
                 wload(iotaR2, "iotaR2"),
                 [wload(iota8_0, "iota8_0"), wload(iota8_1, "iota8_1")],
                 wload(onesb, "onesb"), wload(identB, "identB"),
                 wload(b1d, "b1d"), wload(b2d, "b2d"),
                 wload(b1v, "b1v"), wload(b2v, "b2v"),
                 wload(edb1, "edb1"),
                 wload(KRTm, "KRTm"), wload(KDSm, "KDSm"),
                 wload(I32s4, "I32s4"))
            s_KRTm_, s_KDSm_, s_I32s4_ = r[-3], r[-2], r[-1]
            for i in range(2):
                nc.sync.dma_start(out=R2g[i][0:32, :], in_=s_I32s4_)
                nc.sync.dma_start(
                    out=KRZg[i][32:49, :, 392 + HID:KC],
                    in_=s_KDSm_.unsqueeze(1).broadcast_to([MAXN1, 4, DIM]))
                nc.sync.dma_start(
                    out=KRZg[i][32:49, :, HID:392 + HID],
                    in_=s_KRTm_.unsqueeze(1).broadcast_to([MAXN1, 4, 392]))
            return r

        def load_big_weights():
            return (wload(vdW1, "vdW1"), wload(vdW2, "vdW2"),
                    wload(edW1, "edW1"), wload(edW2, "edW2"),
                    wload(Wze, "Wze"), wload(vmW1x, "vmW1x"),
                    wload(kmW2e, "kmW2e"), wload(vmW2, "vmW2"),
                    wload(emW1y, "emW1y"), wload(EMt, "EMt"),
                    wload(emW2, "emW2"), wload(emOnes, "emOnes"))

        ve = nc.gpsimd if USE_GPS else nc.vector

        def psum(name, shape=(128, NSUB), tag="mmA", bufs=4, dtype=FP):
            return ps.tile(list(shape), dtype, name=name, tag=tag, bufs=bufs)

        def acopy(out, in_):
            nc.scalar.activation(out, in_, AF.Copy)

        def mm_acc(pt, slabs, msl, rhs_parts, extra=None, keep_open=False):
            if not isinstance(slabs, list):
                slabs = [slabs]
            assert len(slabs) == len(rhs_parts)
            nk = len(rhs_parts) + (1 if extra is not None else 0)
            if keep_open:
                nk = len(rhs_parts) + 1
            for i, (sl, rp) in enumerate(zip(slabs, rhs_parts)):
                assert sl.shape[0] == rp.shape[0], (sl.shape, rp.shape)
                lw = sl[:, msl]
                if lw.dtype == FP:
                    lw = lw.bitcast(FPR)
                rr = rp.bitcast(FPR) if rp.dtype == FP else rp
                nc.tensor.matmul(pt, lw, rr, start=(i == 0), stop=(i == nk - 1))
            if extra is not None:
                l2, r2 = extra
                if l2.dtype == FP:
                    l2 = l2.bitcast(FPR)
                if r2.dtype == FP:
                    r2 = r2.bitcast(FPR)
                nc.tensor.matmul(pt, l2, r2, start=False, stop=True)

        x_r = x_d.rearrange("(c q p) d -> c q p d", c=nchunks, p=128)

        st = {}

        def pool_tree_reduce(out2d, pm3):
            tA = work3.tile([128, 32, 8], FP, name="redA", tag="redA")
            tB = work3.tile([128, 32, 4], FP, name="redB", tag="redB")
            tC = work3.tile([128, 32, 2], FP, name="redC", tag="redC")
            ve.tensor_tensor(out=tA, in0=pm3[:, :, 0:8], in1=pm3[:, :, 8:16],
                             op=OP.add)
            ve.tensor_tensor(out=tB, in0=tA[:, :, 0:4], in1=tA[:, :, 4:8],
                             op=OP.add)
            ve.tensor_tensor(out=tC, in0=tB[:, :, 0:2], in1=tB[:, :, 2:4],
                             op=OP.add)
            ve.tensor_tensor(out=out2d.unsqueeze(2), in0=tC[:, :, 0:1],
                             in1=tC[:, :, 1:2], op=OP.add)

        def stage_a(c):
            """Load x, cast to bf16 in DRAM, xbar-transpose to feature-major;
            fp32 rank scores from the original tiles."""
            x_fm = [work.tile([128, CT], BF, name=f"x_fm{h}") for h in (0, 1)]
            mag_tm = work.tile([128, 16], FP, name="mag_tm")
            xbf_d = dstg.tile([CT, DIM], BF, name="xbf_d")
            for hx in range(2):
                x_tm = work.tile([128, 8, DIM], FP, name="x_tm")
                nc.sync.dma_start(
                    out=x_tm,
                    in_=x_r[c, 8 * hx:8 * (hx + 1)].transpose([1, 0, 2]))
                xcast = work.tile([128, 8, DIM], BF, name="xcast")
                ve.tensor_copy(out=xcast, in_=x_tm)
                nc.sync.dma_start(
                    out=xbf_d[CT // 2 * hx:CT // 2 * (hx + 1), :]
                    .rearrange("(q p) d -> p q d", p=128),
                    in_=xcast)
                for nh in range(2):
                    ns = 2 * hx + nh
                    junk = work3.tile([128, DIM], FP, name="junk")
                    for a in range(4):
                        nc.vector.scalar_tensor_tensor(
                            out=junk, in0=x_tm[:, 4 * nh + a, :], scalar=0.0,
                            in1=s_rankWb, op0=OP.bypass, op1=OP.mult,
                            accum_out=mag_tm[:, 4 * ns + a:4 * ns + a + 1])
            for h in range(2):
                nc.sync.dma_start_transpose(
                    out=x_fm[h], in_=xbf_d[:, 128 * h:128 * (h + 1)])
            st[c] = {"x_fm": x_fm, "mag_tm": mag_tm}

        def stage_b1(c):
            """n one-hots + masked mag in position-major layout (gpsimd)."""
            s0 = c * CS
            n_i32 = glob.tile([32, CS], I32, name="n_i32")
            nc.sync.dma_start(
                out=n_i32,
                in_=n_d[s0:s0 + CS].unsqueeze(0).broadcast_to([32, CS]))
            n_repf = glob.tile([32, CS], FP, name="n_repf")
            ve.tensor_copy(out=n_repf, in_=n_i32)
            OHn = glob.tile([32, CS], BF, name="OHn")
            ve.tensor_scalar(
                out=OHn, in0=n_repf, scalar1=s_iota17, scalar2=None,
                op0=OP.is_equal)
            inv = glob.tile([N, CS], FP, name="inv")
            ve.tensor_scalar(
                out=inv, in0=n_repf[0:N], scalar1=s_iota17[0:N], scalar2=None,
                op0=OP.is_le)
            magd = dstg.tile([CT], FP, name="magd")
            nc.sync.dma_start(
                out=magd.rearrange("(q p) -> p q", p=128), in_=st[c]["mag_tm"])
            magT = glob.tile([N, CS], FP, name="magT")
            nc.sync.dma_start(
                out=magT, in_=magd.rearrange("(s i) -> i s", i=N))
            mag_m = glob.tile([N, CS], FP, name="mag_m")
            ve.tensor_scalar(
                out=mag_m, in0=inv, scalar1=BIG, scalar2=None, op0=OP.mult)
            ve.tensor_tensor(
                out=mag_m, in0=mag_m, in1=magT, op=OP.add)
            st[c].update(n_repf=n_repf, OHn=OHn, mag_m=mag_m)

        def stage_b2a(c):
            """Pairwise-comparison ranks -> token-major bf16 ranks in DRAM."""
            mag_m = st[c]["mag_m"]
            n_repf = st[c]["n_repf"]
            pX2 = psum("pX2", (128, CS), tag="sm", bufs=1)
            nc.tensor.matmul(pX2, s_REP16, mag_m)
            X2 = glob.tile([128, CS], FP, name="X2")
            nc.vector.tensor_copy(out=X2, in_=pX2)
            rank_bf = [glob.tile([8, CS], BF, name=f"rank_bf{h}")
                       for h in range(2)]
            for h in range(2):
                pX1 = psum("pX1", (128, CS), tag="sm", bufs=1)
                nc.tensor.matmul(pX1, s_SEL[h], mag_m)
                X1 = glob.tile([128, CS], FP, name="X1")
                nc.vector.tensor_copy(out=X1, in_=pX1)
                cmp = glob.tile([128, CS], BF, name="cmp")
                eq = glob.tile([128, CS], BF, name="eq")
                nc.vector.tensor_tensor(out=cmp, in0=X2, in1=X1, op=OP.is_lt)
                nc.vector.tensor_tensor(out=eq, in0=X2, in1=X1,
                                        op=OP.is_equal)
                ve.tensor_scalar(
                    out=eq, in0=eq, scalar1=s_LT[h], scalar2=None,
                    op0=OP.mult)
                ve.tensor_tensor(
                    out=cmp, in0=cmp, in1=eq, op=OP.add)
                pr = psum("pr", (8, CS), tag="sm", bufs=1)
                nc.tensor.matmul(pr, s_S8, cmp)
                rh = glob.tile([8, CS], FP, name=f"rh{h}")
                nc.vector.tensor_copy(out=rh, in_=pr)
                ih = glob.tile([8, CS], FP, name=f"ih{h}")
                nc.vector.tensor_scalar(
                    out=ih, in0=n_repf[0:8], scalar1=s_iota8[h],
                    scalar2=None, op0=OP.is_le)
                th = glob.tile([8, CS], FP, name=f"th{h}")
                nc.vector.tensor_scalar(
                    out=th, in0=rh, scalar1=-1.0, scalar2=16.0,
                    op0=OP.mult, op1=OP.add)
                nc.vector.tensor_tensor(out=th, in0=th, in1=ih, op=OP.mult)
                nc.vector.tensor_tensor(out=th, in0=th, in1=rh, op=OP.add)
                nc.vector.tensor_copy(out=rank_bf[h], in_=th)
            rankd = dstg.tile([CT], BF, name="rankd")
            for h in range(2):
                nc.sync.dma_start(
                    out=rankd.rearrange("(s i) -> i s", i=N)[8 * h:8 * h + 8],
                    in_=rank_bf[h])
            st[c]["rankd"] = rankd

        def stage_b2b(c):
            """Broadcast token-major ranks into R2g rows 32-49; build the
            rank one-hots (DVE) and the valid-token mask row."""
            R2 = R2g[c % 2]
            mk = maskt[c % 2]
            nc.sync.dma_start(
                out=R2[32:49, :],
                in_=st[c]["rankd"].unsqueeze(0).broadcast_to([MAXN1, CT]))
            nc.vector.tensor_scalar(
                out=R2[32:49, :], in0=R2[32:49, :],
                scalar1=s_iotaR2[32:49], scalar2=None, op0=OP.is_equal)
            nc.sync.dma_start(out=mk, in_=st[c]["rankd"].unsqueeze(0))
            nc.vector.tensor_scalar(
                out=mk, in0=mk, scalar1=15.5, scalar2=None, op0=OP.is_le)

        def phase_1c(c):
            y2ds = [glob.tile([128, CS], BF, name=f"y2ds{m}") for m in (0, 1)]
            x_fm = st[c]["x_fm"]
            R2 = R2g[c % 2]
            KRZ = KRZg[c % 2]
            for ns in range(4):
                tsl = slice(NSUB * ns, NSUB * (ns + 1))
                ssl = slice(32 * ns, 32 * (ns + 1))
                xp = [x_fm[0][:, tsl], x_fm[1][:, tsl]]
                Hd = []
                for m in range(2):
                    pd = psum(f"pd{m}")
                    mm_acc(pd, s_vdW1, slice(128 * m, 128 * (m + 1)), xp)
                    hd = work3.tile([128, NSUB], BF, name=f"Hd{m}")
                    nc.scalar.activation(hd, pd, AF.Relu, bias=s_b1d[m])
                    Hd.append(hd)
                for m in range(2):
                    pg = psum(f"pg{m}", tag="mmB", bufs=2)
                    nc.tensor.matmul(
                        pg,
                        KRZ[32:49, ns, 904 + 128 * m:904 + 128 * (m + 1)],
                        R2[32:49, tsl])
                    kg = work3.tile([128, NSUB], BF, name="KG", tag="KG")
                    acopy(kg, pg)
                    pv = psum(f"pv{m}")
                    mm_acc(pv, s_vdW2, slice(128 * m, 128 * (m + 1)), Hd)
                    pds = work3.tile([128, NSUB], BF, name="Pds", tag="Pds")
                    nc.vector.scalar_tensor_tensor(
                        out=pds, in0=pv, scalar=s_b2d[m],
                        in1=kg, op0=OP.add, op1=OP.mult)
                    pd3 = pds.rearrange("p (s i) -> p s i", i=N)
                    if False:
                        pool_tree_reduce(y2ds[m][:, ssl], pd3)
                    else:
                        nc.vector.tensor_reduce(
                            out=y2ds[m][:, ssl], in_=pd3,
                            axis=AX.X, op=OP.add)
            st[c]["y2ds"] = y2ds

        def phase_2(c):
            y2ds = st[c]["y2ds"]
            KRZ = KRZg[c % 2]
            He = []
            for m in range(2):
                pe = psum(f"pe{m}", (128, CS))
                mm_acc(pe, s_edW1, slice(128 * m, 128 * (m + 1)), y2ds)
                he = glob.tile([128, CS], BF, name=f"He{m}")
                nc.scalar.activation(he, pe, AF.Relu, bias=s_edb1[m])
                He.append(he)
            ze = []
            for m in range(2):
                pz = psum(f"pz{m}", (128, CS), tag="mmB", bufs=2)
                mm_acc(pz, s_edW2, slice(128 * m, 128 * (m + 1)), He)
                z1 = glob.tile([128, CS], BF, name=f"ze{m}")
                acopy(z1, pz)
                ze.append(z1)
            zpT_s = glob.tile([128, 904], BF, name="zpT_s")
            for half in range(2):
                csl = slice(452 * half, 452 * (half + 1))
                pzt = psum("pzt", (128, 452), tag="mmB", bufs=2)
                mm_acc(pzt, [ze[0], ze[1], s_onesb], slice(None),
                       [sw[:, csl] for sw in s_Wze])
                acopy(zpT_s[:, csl], pzt)
            # one contiguous DMA per sub-chunk: zpT rows of the combined lhsT
            for ns in range(4):
                zsl = slice(32 * ns, 32 * (ns + 1))
                nc.sync.dma_start(
                    out=KRZ[0:32, ns, 0:904], in_=zpT_s[zsl, :])

        def phase_3(c):
            y2m = [glob.tile([128, CS], BF, name=f"y2m{m}") for m in range(4)]
            x_fm = st[c]["x_fm"]
            R2 = R2g[c % 2]
            KRZ = KRZg[c % 2]
            for ns in range(4):
                tsl = slice(NSUB * ns, NSUB * (ns + 1))
                ssl = slice(32 * ns, 32 * (ns + 1))
                xp = [x_fm[0][:, tsl], x_fm[1][:, tsl]]
                Hv = []
                for m in range(4):
                    pvm = psum(f"pvm{m}")
                    mm_acc(pvm, s_vmW1x, slice(128 * m, 128 * (m + 1)),
                           xp, keep_open=True)
                    nc.tensor.matmul(
                        pvm, KRZ[0:32, ns, 128 * m:128 * (m + 1)],
                        R2[0:32, tsl], start=False, stop=True)
                    hv = work3.tile([128, NSUB], BF, name=f"Hv{m}")
                    nc.scalar.activation(hv, pvm, AF.Relu, bias=s_b1v[m])
                    Hv.append(hv)
                Hk = []
                for m in range(4):
                    mw = 128 if m < 3 else 8
                    pkm = psum(f"pkm{m}")
                    nc.tensor.matmul(
                        pkm[:mw, :] if mw != 128 else pkm,
                        KRZ[0:49, ns, 512 + 128 * m:512 + 128 * m + mw],
                        R2[0:49, tsl])
                    hk = work3.tile(
                        [mw + 1 if m == 3 else mw, NSUB], BF, name=f"Hk{m}")
                    nc.vector.tensor_scalar(
                        out=hk[:mw, :] if m == 3 else hk,
                        in0=pkm[:mw, :] if mw != 128 else pkm,
                        scalar1=0.0, scalar2=None, op0=OP.max)
                    if m == 3:
                        nc.sync.dma_start(
                            out=hk[mw:mw + 1, :],
                            in_=maskt[c % 2][:, tsl])
                    Hk.append(hk)
                for m in range(4):
                    pK = psum(f"pK{m}", tag="mmB", bufs=2)
                    mm_acc(pK, s_kmW2e, slice(128 * m, 128 * (m + 1)), Hk)
                    km = work3.tile([128, NSUB], BF, name="Km", tag="Km")
                    acopy(km, pK)
                    pV = psum(f"pV{m}")
                    mm_acc(pV, s_vmW2, slice(128 * m, 128 * (m + 1)), Hv)
                    pmt = work3.tile([128, NSUB], BF, name="Pm", tag="Pm")
                    nc.vector.scalar_tensor_tensor(
                        out=pmt, in0=pV, scalar=s_b2v[m],
                        in1=km, op0=OP.add, op1=OP.mult)
                    pm3 = pmt.rearrange("p (s i) -> p s i", i=N)
                    if m < 2 and USE_GPS:
                        pool_tree_reduce(y2m[m][:, ssl], pm3)
                    else:
                        nc.vector.tensor_reduce(
                            out=y2m[m][:, ssl], in_=pm3,
                            axis=AX.X, op=OP.add)
            st[c]["y2m"] = y2m

        def phase_4(c):
            s0 = c * CS
            y2m = st[c]["y2m"]
            OHn = st[c]["OHn"]
            Hm = []
            for m in range(5):
                mw = 128 if m < 4 else 8
                pem = psum(f"pem{m}", (128, CS))
                pem_v = pem[:mw, :] if mw != 128 else pem
                mm_acc(pem_v, s_emW1y, slice(128 * m, 128 * m + mw), y2m,
                       extra=(s_EMt[:, 128 * m:128 * m + mw], OHn[0:MAXN1]))
                hm = glob.tile([mw, CS], BF, name=f"Hm{m}")
                nc.scalar.activation(hm, pem_v, AF.Relu)
                Hm.append(hm)
            zt = psum("zt", (128, 4, 128), tag="tr", bufs=1, dtype=BF)
            for m in range(4):
                pzo = psum(f"pzo{m}", (128, CS), tag="mmB", bufs=2)
                mm_acc(pzo, s_emW2, slice(128 * m, 128 * (m + 1)), Hm,
                       extra=(s_emOnes[:, 128 * m:128 * (m + 1)], s_onesb))
                zo = glob.tile([128, CS], BF, name=f"zo{m}")
                acopy(zo, pzo)
                nc.tensor.transpose(zt[:, m, :], zo, s_identB)
            z_tm = glob.tile([128, 4, 128], FP, name="z_tm")
            nc.vector.tensor_copy(out=z_tm, in_=zt)
            nc.sync.dma_start(
                out=z_d[s0:s0 + CS, :].rearrange("s (m f) -> s m f", m=4),
                in_=z_tm)
            del st[c]

        # ---------------- pipelined emission ----------------
        # software pipeline, two chunks in flight:
        #   iter c emits: A/B1(c+1) | 2(c) | B2(c+1) | 3(c) | 1c(c+1) | 4(c)
        # so the serial ed-MLP / em-MLP chains and the rank machinery of the
        # next chunk overlap the dense matmul phases of the current one.
        (s_rankWb, s_iota17) = load_small_early()
        stage_a(0)
        stage_b1(0)
        (s_REP16, s_SEL, s_S8, s_LT, s_iotaR2, s_iota8, s_onesb, s_identB,
         s_b1d, s_b2d, s_b1v, s_b2v, s_edb1, s_KRTm, s_KDSm,
         s_I32s4) = load_small_rest()
        (s_vdW1, s_vdW2, s_edW1, s_edW2, s_Wze, s_vmW1x, s_kmW2e,
         s_vmW2, s_emW1y, s_EMt, s_emW2, s_emOnes) = load_big_weights()
        stage_b2a(0)
        stage_b2b(0)
        phase_1c(0)
        for c in range(nchunks):
            if c + 1 < nchunks:
                stage_a(c + 1)
                stage_b1(c + 1)
            phase_2(c)
            if c + 1 < nchunks:
                stage_b2a(c + 1)
                stage_b2b(c + 1)
            phase_3(c)
            if c + 1 < nchunks:
                phase_1c(c + 1)
            phase_4(c)

    nc.compile()
    return nc


def make_tables(inp):
    """Host-side weight preprocessing -> dict of extra input arrays."""
    f = np.float32
    import ml_dtypes

    def BFC(v):
        return np.ascontiguousarray(
            np.asarray(v, f).astype(ml_dtypes.bfloat16))

    keys = ("rank_W", "kd_W1", "kd_b1", "kd_W2", "kd_b2", "vd_W1", "vd_b1",
            "vd_W2", "vd_b2", "ed_W1", "ed_b1", "ed_W2", "ed_b2", "km_W1",
            "km_b1", "km_W2", "km_b2", "vm_W1", "vm_b1", "vm_W2", "vm_b2",
            "em_W1", "em_b1", "em_W2", "em_b2")
    g = {k: np.asarray(inp[k], f) for k in keys}

    def A(v):
        return np.ascontiguousarray(v, dtype=f)

    kd_h = np.maximum(g["kd_W1"][:16] + g["kd_b1"][None, :], 0.0)
    Kds16 = kd_h @ g["kd_W2"] + g["kd_b2"][None, :]
    Kds17 = np.vstack([Kds16, np.zeros((1, DIM), f)])
    KRt17 = np.vstack([g["km_W1"][:16] + g["km_b1"][None, :],
                       np.full((1, 392), -BIG, f)])

    kmW2e = np.vstack([g["km_W2"], g["km_b2"][None, :]])

    Wz = np.hstack([g["vm_W1"][DIM:2 * DIM], g["km_W1"][MAXN1:MAXN1 + DIM]])
    Wze = np.vstack([Wz, (g["ed_b2"] @ Wz)[None, :]])

    EMt = g["em_W1"][HID:HID + MAXN1] + g["em_b1"][None, :]

    p = np.arange(128)
    t = np.arange(NSUB)
    i32g = (t[None, :] // N == np.arange(32)[:, None]).astype(f)
    I32s4 = np.tile(i32g, (1, 4))
    S8 = (p[:, None] // 16 == np.arange(8)[None, :]).astype(f)
    LT0 = ((p % 16) < (p // 16)).astype(f)[:, None]
    LT1 = ((p % 16) < (p // 16 + 8)).astype(f)[:, None]
    REP16 = (np.arange(16)[:, None] == (p % 16)[None, :]).astype(f)
    SEL0 = (np.arange(16)[:, None] == (p // 16)[None, :]).astype(f)
    SEL1 = (np.arange(16)[:, None] == (p // 16 + 8)[None, :]).astype(f)
    iotaR2 = np.full((50, 1), 99.0, f)
    iotaR2[32:49, 0] = np.arange(MAXN1)

    return {
        "vdW1": A(g["vd_W1"]), "b1d": A(g["vd_b1"][:, None]),
        "vdW2": A(g["vd_W2"]), "b2d": A(g["vd_b2"][:, None]),
        "vmW1x": A(g["vm_W1"][:DIM]), "b1v": A(g["vm_b1"][:, None]),
        "vmW2": A(g["vm_W2"]), "b2v": A(g["vm_b2"][:, None]),
        "kmW2e": A(kmW2e),
        "edW1": A(g["ed_W1"]), "edb1": A(g["ed_b1"][:, None]),
        "edW2": A(g["ed_W2"]), "Wze": A(Wze),
        "emW1y": A(g["em_W1"][:HID]), "EMt": A(EMt),
        "emW2": A(g["em_W2"]),
        "emOnes": A(g["em_b2"][None, :]),
        "rankWb": A(np.tile(g["rank_W"].T, (128, 1))),
        "identB": BFC(np.eye(128)),
        "KRTm": BFC(KRt17), "KDSm": BFC(Kds17),
        "I32s4": BFC(I32s4),
        "REP16": A(REP16), "SEL0": A(SEL0), "SEL1": A(SEL1),
        "S8": BFC(S8), "LT0": A(LT0), "LT1": A(LT1),
        "iotaR2": A(iotaR2),
        "iota17": A(np.concatenate(
            [np.arange(MAXN1), np.full(32 - MAXN1, 99.0)])[:, None]),
        "iota8_0": A(np.arange(8)[:, None]),
        "iota8_1": A(np.arange(8, 16)[:, None]),
        "onesb": BFC(np.ones((1, CS))),
    }


_prog_cache = {}


def _get_program(sets_per_core, num_devices):
    key = (sets_per_core, num_devices)
    if key not in _prog_cache:
        _prog_cache[key] = build_program(sets_per_core, num_devices)
    return _prog_cache[key]


def kernel(**inputs):
    nc = _get_program(SC, NCORES)
    tabs = make_tables(inputs)
    x = np.ascontiguousarray(np.asarray(inputs["x"], np.float32))
    n = np.ascontiguousarray(np.asarray(inputs["n"], np.int32))
    in_maps = []
    for c in range(NCORES):
        m = dict(tabs)
        m["x"] = np.ascontiguousarray(
            x[c * SC:(c + 1) * SC].reshape(SC * N, DIM))
        m["n_i"] = n[c * SC:(c + 1) * SC]
        in_maps.append(m)
    res = bass_utils.run_bass_kernel_spmd(nc, in_maps, list(range(NCORES)))
    z = np.concatenate([res.results[c]["z_out"] for c in range(NCORES)], 0)
    return z
